# revision 30
# baseline (speedup 1.0000x reference)
"""DeepONet-style neural operator forward pass on 8 TRN2 NeuronCores.

Pure data parallel over the batch (131072 rows -> 16384/core), weights
replicated, activations feature-major ([feat, rows]), 512-row blocks.

Key layout trick: the host emits a single fp16 matrix stac16w [rows, 768]
whose columns are (a) the 21 feature/aux values (pos, ones, state, action,
pos^2) and (b) all 544 sensor-replicated enc channels (j-major) plus an
ones bias channel. Each block issues 6 XBAR DMA-transposes (one per
128-column chunk) straight into SBUF, so the PE never transposes inputs
and never runs replication matmuls. Sensor dist^2 is one K=21 fp16 matmul
(|s|^2, |pos|^2 folded via ones/pos^2 rows), sqrt is a fp16 magic-rsqrt +
1 Newton step, exp on ACT, and the enc = srep * w multiplies run on the
otherwise-idle GPSIMD engine (SBUF-only). All matmuls are fp16 (1 col/cyc;
fp8 DoubleRow is power-throttled to a 50% duty cycle on this part, so it
buys nothing sustained). L1 bias is folded via the ones channel; tt/qnet
biases via ones rows; bb2/bb3/tb2 via per-partition bias pointers.
"""

import numpy as np

import concourse.bass as bass
import concourse.mybir as mybir
import concourse.tile as tile
from concourse import bacc

F32 = mybir.dt.float32
F16 = mybir.dt.float16
I16 = mybir.dt.int16
I32 = mybir.dt.int32
AF = mybir.ActivationFunctionType
ALU = mybir.AluOpType
AX = mybir.AxisListType

SD = 13          # state dim
AD = 4           # action dim
J = SD + AD      # 17 per-sensor features
NS = 32          # sensors
H1, H2, H4, H8 = 1024, 512, 256, 128
B_FULL = 131072
N_CORES = 8
RPC = B_FULL // N_CORES   # rows per core
NB = 512                  # rows per block (= fp32 PSUM bank)
WC = 768                  # stac16w columns (6 chunks of 128)

# stacT row layout (chunk 0): 0-2 pos, 3 ones, 4-13 state[3:], 14-17 action,
# 18-20 pos^2.  Chunks 1-4: enc channels 0..511 (ch = j*32+s).  Chunk 5:
# rows 0-31 = enc channels 512..543, row 32 = ones (L1 bias channel).
ROWMAP = [j if j < 3 else j + 1 for j in range(J)]

# engine split for the relus after L1 (8 m-tiles) and L2 (4 m-tiles)
RELU1_ENG = ["act"] * 8
RELU2_ENG = ["vector"] * 4


def _const_specs():
    b16 = []
    for k in range(4):
        b16.append((f"w1_{k}", 128, H1))
    b16.append(("w1_4", 33, H1))          # 32 enc channels + bias row
    for k in range(8):
        b16.append((f"w2_{k}", 128, H2))
    for k in range(4):
        b16.append((f"w3_{k}", 128, H4))
    for k in range(2):
        b16.append((f"tw2_{k}", 128, H4))
    b16 += [("pw_0", 128, SD), ("pw_1", 128, SD), ("qw2", H8, SD),
            ("sl21", 21, 128), ("tw1b", 4, 256), ("qw1b", 4, 128),
            ("id13h", SD, SD)]
    bf = [("bb2t", 128, 4), ("bb3t", 128, 2), ("tb2t", 128, 2),
          ("c13", SD, 1), ("rw13", SD, 1), ("id13", SD, SD)]

    def offsets(specs):
        out, o = {}, 0
        for name, p, w in specs:
            out[name] = (o, p, w)
            o += w
        return out, o
    o16, w16 = offsets(b16)
    of, wf = offsets(bf)
    return o16, w16, of, wf


C16, C16W, CF, CFW = _const_specs()


def build_nc(rpc=RPC, repeats=1):
    assert rpc % NB == 0
    nblk = rpc // NB
    nc = bacc.Bacc(trn_type="TRN2")

    def inp(name, shape, dt=F32):
        return nc.dram_tensor(name, shape, dt, kind="ExternalInput").ap()

    state = inp("state", [rpc, SD])
    stac16w = inp("stac16w", [rpc, WC], F16)
    blob16 = inp("blob16", [128, C16W], F16)
    blobf = inp("blobf", [128, CFW])

    out = nc.dram_tensor("out", [rpc, SD], F32, kind="ExternalOutput").ap()

    with tile.TileContext(nc) as tc:
        for _rep in range(repeats):
            _body(tc, nblk, locals())
    nc.compile()
    return nc


def _body(tc, nblk, t):
    nc = tc.nc

    import contextlib
    stack = contextlib.ExitStack()
    consts = stack.enter_context(tc.tile_pool(name="consts", bufs=1))
    sb_in = stack.enter_context(tc.tile_pool(name="sb_in", bufs=1))
    sb_act = stack.enter_context(tc.tile_pool(name="sb_act", bufs=1))
    sb_sm = stack.enter_context(tc.tile_pool(name="sb_sm", bufs=1))
    ps_mm = stack.enter_context(tc.tile_pool(name="ps_mm", bufs=7,
                                             space="PSUM"))
    ps_tr = stack.enter_context(tc.tile_pool(name="ps_tr", bufs=1,
                                             space="PSUM"))

    blob16_sb = consts.tile([128, C16W], F16, name="blob16_sb",
                            tag="blob16_sb")
    blobf_sb = consts.tile([128, CFW], F32, name="blobf_sb", tag="blobf_sb")
    NCH = 8
    step = (C16W + NCH - 1) // NCH
    for i in range(NCH):
        a, b = i * step, min((i + 1) * step, C16W)
        nc.scalar.dma_start(out=blob16_sb[:, a:b], in_=t["blob16"][:, a:b])
    nc.scalar.dma_start(out=blobf_sb, in_=t["blobf"])

    def v16(name):
        o, p, w = C16[name]
        return blob16_sb[0:p, o:o + w]

    def vf(name):
        o, p, w = CF[name]
        return blobf_sb[0:p, o:o + w]

    w1sb = [v16(f"w1_{k}") for k in range(5)]
    w2sb = [v16(f"w2_{k}") for k in range(8)]
    w3sb = [v16(f"w3_{k}") for k in range(4)]
    tw2sb = [v16(f"tw2_{k}") for k in range(2)]
    pwsb = [v16("pw_0"), v16("pw_1")]
    qw2sb = v16("qw2")
    sl21 = v16("sl21")
    tw1b = v16("tw1b")
    qw1b = v16("qw1b")
    id13h = v16("id13h")
    bb2sb = vf("bb2t")
    bb3sb = vf("bb3t")
    tb2sb = vf("tb2t")
    c13sb = vf("c13")
    rw13sb = vf("rw13")
    id13sb = vf("id13")
    zero1 = consts.tile([128, 1], F32)
    nc.vector.memset(zero1, 0.0)

    state, stac16w, outdr = t["state"], t["stac16w"], t["out"]

    # PE p-state warmup: dummy matmuls on a zeroed tile while the first
    # input DMAs land, so block 0 starts at full clock.
    wu_l = consts.tile([4, 128], F16, name="wu_l")
    wu_r = consts.tile([4, NB], F16, name="wu_r")
    nc.vector.memset(wu_l, 0.0)
    nc.vector.memset(wu_r, 0.0)
    wu_ps = ps_mm.tile([128, NB], F32, tag="mm", bufs=7)
    for _ in range(40):
        nc.tensor.matmul(wu_ps, wu_l, wu_r, start=True, stop=True)

    ablk = {}

    def stage_a(blk):
        r0 = blk * NB
        st_ac = sb_in.tile([128, 4, SD], F32, tag="st_ac", bufs=4)
        nc.sync.dma_start(
            out=st_ac,
            in_=state[r0:r0 + NB, :].rearrange("(c p) d -> p c d", p=128))
        # 6 XBAR transposes: chunk 0 = features, 1-5 = enc channel groups
        stacT = sb_in.tile([128, NB], F16, tag="stacT", bufs=3)
        nc.sync.dma_start(out=stacT, in_=stac16w[r0:r0 + NB, 0:128],
                          transpose=True)
        srep = []
        for c in range(5):
            s_ = sb_in.tile([128, NB], F16, tag=f"srep{c}", bufs=3,
                            name=f"srep{c}")
            eng = nc.scalar if c % 2 else nc.sync
            eng.dma_start(
                out=s_,
                in_=stac16w[r0:r0 + NB, 128 * (c + 1):128 * (c + 2)],
                transpose=True)
            srep.append(s_)

        # q = dist^2 (K=21 fp16 matmul; |s|^2, |pos|^2 folded via const rows)
        q_ps = ps_mm.tile([128, NB], F32, tag="mm", bufs=7)
        nc.tensor.matmul(q_ps, sl21, stacT[0:21, :], start=True, stop=True)
        qs = sb_sm.tile([128, NB], F16, tag="qs", bufs=3)
        nc.scalar.activation(out=qs, in_=q_ps, func=AF.Relu,
                             bias=zero1[:, 0:1], scale=1.0)

        # dist = q * rsqrt(q): fp16 magic seed + 1 Newton step
        r = sb_sm.tile([128, NB], F16, tag="r", bufs=3)
        y = sb_sm.tile([128, NB], F16, tag="y", bufs=3)
        u = sb_sm.tile([128, NB], F16, tag="u", bufs=3)
        nc.vector.tensor_scalar(
            out=r.bitcast(I16), in0=qs.bitcast(I16), scalar1=1, scalar2=None,
            op0=ALU.logical_shift_right)
        nc.vector.tensor_scalar(
            out=r.bitcast(I16), in0=r.bitcast(I16), scalar1=-1,
            scalar2=0x59BA, op0=ALU.mult, op1=ALU.add)
        nc.gpsimd.tensor_mul(y, qs, r)
        nc.gpsimd.tensor_mul(u, y, r)
        nc.gpsimd.tensor_scalar(out=u, in0=u, scalar1=-0.5, scalar2=1.5,
                                op0=ALU.mult, op1=ALU.add)
        nc.gpsimd.tensor_mul(y, y, u)   # y = dist

        w16 = sb_in.tile([128, NB], F16, tag="w16", bufs=3)
        nc.scalar.activation(out=w16, in_=y, func=AF.Exp,
                             bias=zero1[:, 0:1], scale=-2.0)

        # enc = srep * w[p%32] on gpsimd (SBUF only); chunk 5: rows 0-31
        # are channels (*w), row 32 is the ones bias channel (copied).
        enc = []
        for c in range(4):
            et = sb_in.tile([128, NB], F16, tag=f"enc{c}", bufs=3,
                            name=f"enc{c}")
            nc.gpsimd.tensor_mul(et, srep[c], w16)
            enc.append(et)
        etC = sb_in.tile([33, NB], F16, tag="encC", bufs=3)
        nc.gpsimd.tensor_mul(etC[0:32, :], srep[4][0:32, :], w16[0:32, :])
        nc.vector.tensor_copy(etC[32:33, :], srep[4][32:33, :])
        enc.append(etC)
        ablk[blk] = dict(st_ac=st_ac, stacT=stacT, enc=enc)

    def stage_b(blk):
        st = ablk[blk]
        enc, stacT = st["enc"], st["stacT"]

        def relu_to(eng, dst, ps, bias_col=None):
            if eng == "act":
                nc.scalar.activation(
                    out=dst, in_=ps, func=AF.Relu,
                    bias=zero1[:, 0:1] if bias_col is None else bias_col,
                    scale=1.0)
            elif bias_col is None:
                nc.vector.tensor_scalar_max(dst, ps, 0.0)
            else:
                nc.vector.tensor_scalar(out=dst, in0=ps, scalar1=bias_col,
                                        scalar2=0.0, op0=ALU.add,
                                        op1=ALU.max)

        # ---- branch L1: 544 -> 1024 (5 fp16 chunks; bias pre-folded) ----
        h1 = [sb_act.tile([128, NB], F16, tag=f"h1_{m}", bufs=2,
                          name=f"h1_{m}") for m in range(8)]
        for m in range(8):
            ps = ps_mm.tile([128, NB], F32, tag="mm", bufs=7)
            for k in range(5):
                nc.tensor.matmul(ps, w1sb[k][:, m * 128:(m + 1) * 128],
                                 enc[k], start=(k == 0), stop=(k == 4))
            relu_to(RELU1_ENG[m], h1[m], ps)

        # ---- branch L2: 1024 -> 512 ----
        h2 = [sb_act.tile([128, NB], F16, tag=f"h2_{m}", bufs=2,
                          name=f"h2_{m}") for m in range(4)]
        for m in range(4):
            ps = ps_mm.tile([128, NB], F32, tag="mm", bufs=7)
            for k in range(8):
                nc.tensor.matmul(ps, w2sb[k][:, m * 128:(m + 1) * 128],
                                 h1[k], start=(k == 0), stop=(k == 7))
            relu_to(RELU2_ENG[m], h2[m], ps, bb2sb[:, m:m + 1])

        # ---- trunk: tanh(pos@tw1+tb1) [bias folded], tanh(.@tw2+tb2) ----
        tt = sb_act.tile([128, 2, NB], F16, tag="tt", bufs=2)
        for i_ in range(2):
            tt_ps = ps_mm.tile([128, NB], F32, tag="mm", bufs=7)
            nc.tensor.matmul(tt_ps, tw1b[:, 128 * i_:128 * (i_ + 1)],
                             stacT[0:4, :], start=True, stop=True)
            nc.scalar.activation(out=tt[:, i_, :], in_=tt_ps, func=AF.Tanh,
                                 bias=zero1[:, 0:1], scale=1.0)
        trunk = []
        for m in range(2):
            ps = ps_mm.tile([128, NB], F32, tag="mm", bufs=7)
            for k in range(2):
                nc.tensor.matmul(ps, tw2sb[k][:, m * 128:(m + 1) * 128],
                                 tt[:, k, :], start=(k == 0), stop=(k == 1))
            tm = sb_act.tile([128, NB], F16, tag="trunk", bufs=3)
            nc.scalar.activation(out=tm, in_=ps, func=AF.Tanh,
                                 bias=tb2sb[:, m:m + 1], scale=1.0)
            trunk.append(tm)

        # ---- qnet hidden: relu(pos@qw1+qb1) [bias folded] ----
        ps = ps_mm.tile([128, NB], F32, tag="mm", bufs=7)
        nc.tensor.matmul(ps, qw1b, stacT[0:4, :], start=True, stop=True)
        bq = sb_act.tile([128, NB], F16, tag="bq", bufs=2)
        nc.scalar.activation(out=bq, in_=ps, func=AF.Relu,
                             bias=zero1[:, 0:1], scale=1.0)

        # ---- branch L3 (+bias) fused with interaction multiply ----
        inter = []
        for m in range(2):
            ps = ps_mm.tile([128, NB], F32, tag="mm", bufs=7)
            for k in range(4):
                nc.tensor.matmul(ps, w3sb[k][:, m * 128:(m + 1) * 128],
                                 h2[k], start=(k == 0), stop=(k == 3))
            im = sb_act.tile([128, NB], F16, tag=f"inter{m}", bufs=2,
                             name=f"inter{m}")
            nc.vector.scalar_tensor_tensor(
                out=im, in0=ps, scalar=bb3sb[:, m:m + 1], in1=trunk[m],
                op0=ALU.add, op1=ALU.mult)
            inter.append(im)

        # ---- tail: delta^T + bias_out^T accumulated in one psum ----
        tail_full = ps_mm.tile([128, NB], F32, tag="mm", bufs=7)
        tail_ps = tail_full[0:SD, :]
        nc.tensor.matmul(tail_ps, pwsb[0], inter[0], start=True, stop=False)
        nc.tensor.matmul(tail_ps, pwsb[1], inter[1], start=False, stop=False)
        nc.tensor.matmul(tail_ps, qw2sb, bq, start=False, stop=True)
        combT = sb_sm.tile([16, NB], F16, tag="combT", bufs=3)
        nc.vector.tensor_scalar(
            out=combT[0:SD, :], in0=tail_ps, scalar1=rw13sb[:, 0:1],
            scalar2=c13sb[:, 0:1], op0=ALU.mult, op1=ALU.add)
        ablk[blk]["combT"] = combT

    def stage_c(blk):
        r0 = blk * NB
        st = ablk.pop(blk)
        st_ac, combT = st["st_ac"], st["combT"]
        trt = ps_tr.tile([128, 64], F16, tag="tr", bufs=1)
        nxt = sb_sm.tile([128, 4, SD], F32, tag="nxt", bufs=2)
        sq = sb_sm.tile([128, 4, 4], F32, tag="sq", bufs=2)
        for c in range(4):
            tr_ps = trt[:, 16 * c:16 * c + SD]
            nc.tensor.transpose(tr_ps, combT[0:SD, c * 128:(c + 1) * 128],
                                id13h)
            nc.vector.tensor_add(nxt[:, c, :], tr_ps, st_ac[:, c, :])
            nc.vector.tensor_mul(sq[:, c, :], nxt[:, c, 3:7], nxt[:, c, 3:7])
        qn = sb_sm.tile([128, 4], F32, tag="qn", bufs=2)
        nc.vector.reduce_sum(out=qn.rearrange("p (c o) -> p c o", o=1),
                             in_=sq, axis=AX.X)
        rq = sb_sm.tile([128, 4], F32, tag="rq", bufs=2)
        uq = sb_sm.tile([128, 4], F32, tag="uq", bufs=2)
        yq = sb_sm.tile([128, 4], F32, tag="yq", bufs=2)
        nc.vector.tensor_scalar(
            out=rq.bitcast(I32), in0=qn.bitcast(I32), scalar1=1, scalar2=None,
            op0=ALU.arith_shift_right)
        nc.vector.tensor_scalar(
            out=rq.bitcast(I32), in0=rq.bitcast(I32), scalar1=-1,
            scalar2=0x5F3759DF, op0=ALU.mult, op1=ALU.add)
        for it in range(2):
            nc.gpsimd.tensor_mul(yq, qn, rq)
            nc.gpsimd.tensor_mul(uq, yq, rq)
            nc.gpsimd.tensor_scalar(out=uq, in0=uq, scalar1=-0.5, scalar2=1.5,
                                    op0=ALU.mult, op1=ALU.add)
            nc.gpsimd.tensor_mul(rq, rq, uq)
        outt = sb_sm.tile([128, 4, SD], F32, tag="outt", bufs=2)
        nc.gpsimd.tensor_copy(outt, nxt)
        for c in range(4):
            nc.gpsimd.tensor_scalar_mul(
                outt[:, c, 3:7], nxt[:, c, 3:7], rq[:, c:c + 1])
        out_dst = outdr[r0:r0 + NB, :].rearrange("(c p) d -> p c d", p=128)
        nc.sync.dma_start(out=out_dst, in_=outt)

    # software-pipelined emission: A two blocks ahead of B/C
    stage_a(0)
    if nblk > 1:
        stage_a(1)
    for blk in range(nblk):
        stage_b(blk)
        if blk + 2 < nblk:
            stage_a(blk + 2)
        stage_c(blk)
    stack.close()


def _host_prep(inputs):
    """Precompute fp16 weight blob and the wide replicated feature matrix."""
    f = lambda x: np.ascontiguousarray(np.asarray(x, dtype=np.float32))
    sl = f(inputs["sensor_locations"])            # [32, 3]
    pidx = np.arange(128) % NS

    sl21 = np.zeros((21, 128), np.float32)
    sl21[0:3, :] = -2.0 * sl[pidx].T
    sl21[3, :] = np.square(sl).sum(1)[pidx]
    sl21[18:21, :] = 1.0
    tw1b = np.concatenate([f(inputs["tw1"]), f(inputs["tb1"])[None, :]], 0)
    qw1b = np.concatenate([f(inputs["qw1"]), f(inputs["qb1"])[None, :]], 0)

    # enc channel ch = j*32 + s  <-  original bw1 row s*17 + j
    ch = np.arange(544)
    w1p = f(inputs["bw1"])[(ch % NS) * J + ch // NS, :]        # [544, 1024]

    c16 = {"sl21": sl21, "tw1b": tw1b, "qw1b": qw1b,
           "id13h": np.eye(SD, dtype=np.float32)}
    for k in range(4):
        c16[f"w1_{k}"] = w1p[k * 128:(k + 1) * 128]
    w1c = np.zeros((33, H1), np.float32)
    w1c[0:32] = w1p[512:544]
    w1c[32] = f(inputs["bb1"])
    c16["w1_4"] = w1c
    w2 = f(inputs["bw2"]); w3 = f(inputs["bw3"]); tw2 = f(inputs["tw2"])
    for k in range(8):
        c16[f"w2_{k}"] = w2[k * 128:(k + 1) * 128]
    for k in range(4):
        c16[f"w3_{k}"] = w3[k * 128:(k + 1) * 128]
    for k in range(2):
        c16[f"tw2_{k}"] = tw2[k * 128:(k + 1) * 128]
    pw = f(inputs["pw"])
    c16["pw_0"] = pw[0:128]
    c16["pw_1"] = pw[128:256]
    c16["qw2"] = f(inputs["qw2"])

    def tb(b, nm):
        return np.ascontiguousarray(f(b).reshape(nm, 128).T)

    rw = np.float32(np.asarray(inputs["residual_weight"]))
    cf = {
        "bb2t": tb(inputs["bb2"], 4), "bb3t": tb(inputs["bb3"], 2),
        "tb2t": tb(inputs["tb2"], 2),
        "c13": (rw * (f(inputs["pb"]) + f(inputs["qb2"]))).reshape(SD, 1),
        "rw13": np.full((SD, 1), rw, np.float32),
        "id13": np.eye(SD, dtype=np.float32),
    }

    blob16 = np.zeros((128, C16W), np.float16)
    for name, (o, p, w) in C16.items():
        blob16[0:p, o:o + w] = c16[name].astype(np.float16)
    blobf = np.zeros((128, CFW), np.float32)
    for name, (o, p, w) in CF.items():
        blobf[0:p, o:o + w] = cf[name]

    # stac16w: fp16 features + host-replicated enc channels
    st = f(inputs["state"]); ac = f(inputs["action"])
    B = st.shape[0]
    feat = np.zeros((B, 21), np.float32)
    feat[:, 0:3] = st[:, 0:3]
    feat[:, 3] = 1.0
    feat[:, 4:14] = st[:, 3:13]
    feat[:, 14:18] = ac
    feat[:, 18:21] = np.square(st[:, 0:3])
    stac16w = np.zeros((B, WC), np.float16)
    stac16w[:, 0:21] = feat
    # channels ch = j*32+s -> column 128+ch; value = feature j
    jvals = np.concatenate([st, ac], axis=1).astype(np.float16)  # [B, 17]
    stac16w[:, 128:672] = np.repeat(jvals, NS, axis=1)
    stac16w[:, 672] = 1.0
    return dict(blob16=blob16, blobf=blobf), stac16w


def _core_inputs(inputs, common=None):
    """Build the 8 per-core input maps from the full problem inputs."""
    if common is None:
        common, stac16w = _host_prep(inputs)
    else:
        common, stac16w = common
    state = np.ascontiguousarray(np.asarray(inputs["state"], np.float32))
    in_maps = []
    for i in range(N_CORES):
        m = dict(common)
        m["state"] = state[i * RPC:(i + 1) * RPC]
        m["stac16w"] = stac16w[i * RPC:(i + 1) * RPC]
        in_maps.append(m)
    return in_maps


_NC_CACHE = {}


def _get_nc(rpc=RPC):
    if rpc not in _NC_CACHE:
        _NC_CACHE[rpc] = build_nc(rpc)
    return _NC_CACHE[rpc]


def kernel(**inputs):
    from concourse.bass_utils import run_bass_kernel_spmd

    nc = _get_nc()
    in_maps = _core_inputs(inputs)
    res = run_bass_kernel_spmd(nc, in_maps, list(range(N_CORES)))
    return np.concatenate([r["out"] for r in res.results], axis=0)


# revision 31
# speedup vs baseline: 1.1448x; 1.1448x over previous
"""DeepONet-style neural operator forward pass on 8 TRN2 NeuronCores.

Pure data parallel over the batch (131072 rows -> 16384/core), weights
replicated, activations feature-major ([feat, rows]), 512-row blocks.

Key layout trick: the host emits a single fp16 matrix stac16w [rows, 768]
whose columns are (a) the 21 feature/aux values (pos, ones, state, action,
pos^2) and (b) all 544 sensor-replicated enc channels (j-major) plus an
ones bias channel. Each block issues 6 XBAR DMA-transposes (one per
128-column chunk) straight into SBUF, so the PE never transposes inputs
and never runs replication matmuls. Sensor dist^2 is one K=21 fp16 matmul
(|s|^2, |pos|^2 folded via ones/pos^2 rows), sqrt is a fp16 magic-rsqrt +
1 Newton step, exp on ACT, and the enc = srep * w multiplies run on the
otherwise-idle GPSIMD engine (SBUF-only). All matmuls are fp16 (1 col/cyc;
fp8 DoubleRow is power-throttled to a 50% duty cycle on this part, so it
buys nothing sustained). L1 bias is folded via the ones channel; tt/qnet
biases via ones rows; bb2/bb3/tb2 via per-partition bias pointers.
"""

import numpy as np

import concourse.bass as bass
import concourse.mybir as mybir
import concourse.tile as tile
from concourse import bacc

F32 = mybir.dt.float32
F16 = mybir.dt.float16
I16 = mybir.dt.int16
I32 = mybir.dt.int32
AF = mybir.ActivationFunctionType
ALU = mybir.AluOpType
AX = mybir.AxisListType

SD = 13          # state dim
AD = 4           # action dim
J = SD + AD      # 17 per-sensor features
NS = 32          # sensors
H1, H2, H4, H8 = 1024, 512, 256, 128
B_FULL = 131072
N_CORES = 8
RPC = B_FULL // N_CORES   # rows per core
NB = 512                  # rows per block (= fp32 PSUM bank)
WC = 768                  # stac16w columns (6 chunks of 128)

# stacT row layout (chunk 0): 0-2 pos, 3 ones, 4-13 state[3:], 14-17 action,
# 18-20 pos^2.  Chunks 1-4: enc channels 0..511 (ch = j*32+s).  Chunk 5:
# rows 0-31 = enc channels 512..543, row 32 = ones (L1 bias channel).
ROWMAP = [j if j < 3 else j + 1 for j in range(J)]

# engine split for the relus after L1 (8 m-tiles) and L2 (4 m-tiles)
RELU1_ENG = ["act", "vector", "act", "vector", "act", "vector", "act",
             "vector"]
RELU2_ENG = ["vector", "act", "vector", "act"]


def _const_specs():
    b16 = []
    for k in range(4):
        b16.append((f"w1_{k}", 128, H1))
    b16.append(("w1_4", 33, H1))          # 32 enc channels + bias row
    for k in range(8):
        b16.append((f"w2_{k}", 128, H2))
    for k in range(4):
        b16.append((f"w3_{k}", 128, H4))
    for k in range(2):
        b16.append((f"tw2_{k}", 128, H4))
    b16 += [("pw_0", 128, SD), ("pw_1", 128, SD), ("qw2", H8, SD),
            ("sl21", 21, 128), ("tw1b", 4, 256), ("qw1b", 4, 128),
            ("id13h", SD, SD)]
    bf = [("bb2t", 128, 4), ("bb3t", 128, 2), ("tb2t", 128, 2),
          ("c13", SD, 1), ("rw13", SD, 1), ("id13", SD, SD)]

    def offsets(specs):
        out, o = {}, 0
        for name, p, w in specs:
            out[name] = (o, p, w)
            o += w
        return out, o
    o16, w16 = offsets(b16)
    of, wf = offsets(bf)
    return o16, w16, of, wf


C16, C16W, CF, CFW = _const_specs()


def build_nc(rpc=RPC, repeats=1):
    assert rpc % NB == 0
    nblk = rpc // NB
    nc = bacc.Bacc(trn_type="TRN2")

    def inp(name, shape, dt=F32):
        return nc.dram_tensor(name, shape, dt, kind="ExternalInput").ap()

    state = inp("state", [rpc, SD])
    stac16w = inp("stac16w", [rpc, WC], F16)
    blob16 = inp("blob16", [128, C16W], F16)
    blobf = inp("blobf", [128, CFW])

    out = nc.dram_tensor("out", [rpc, SD], F32, kind="ExternalOutput").ap()

    with tile.TileContext(nc) as tc:
        for _rep in range(repeats):
            _body(tc, nblk, locals())
    nc.compile()
    return nc


def _body(tc, nblk, t):
    nc = tc.nc

    import contextlib
    stack = contextlib.ExitStack()
    consts = stack.enter_context(tc.tile_pool(name="consts", bufs=1))
    sb_in = stack.enter_context(tc.tile_pool(name="sb_in", bufs=1))
    sb_act = stack.enter_context(tc.tile_pool(name="sb_act", bufs=1))
    sb_sm = stack.enter_context(tc.tile_pool(name="sb_sm", bufs=1))
    ps_mm = stack.enter_context(tc.tile_pool(name="ps_mm", bufs=7,
                                             space="PSUM"))
    ps_tr = stack.enter_context(tc.tile_pool(name="ps_tr", bufs=1,
                                             space="PSUM"))

    blob16_sb = consts.tile([128, C16W], F16, name="blob16_sb",
                            tag="blob16_sb")
    blobf_sb = consts.tile([128, CFW], F32, name="blobf_sb", tag="blobf_sb")
    NCH = 8
    step = (C16W + NCH - 1) // NCH
    for i in range(NCH):
        a, b = i * step, min((i + 1) * step, C16W)
        nc.scalar.dma_start(out=blob16_sb[:, a:b], in_=t["blob16"][:, a:b])
    nc.scalar.dma_start(out=blobf_sb, in_=t["blobf"])

    def v16(name):
        o, p, w = C16[name]
        return blob16_sb[0:p, o:o + w]

    def vf(name):
        o, p, w = CF[name]
        return blobf_sb[0:p, o:o + w]

    w1sb = [v16(f"w1_{k}") for k in range(5)]
    w2sb = [v16(f"w2_{k}") for k in range(8)]
    w3sb = [v16(f"w3_{k}") for k in range(4)]
    tw2sb = [v16(f"tw2_{k}") for k in range(2)]
    pwsb = [v16("pw_0"), v16("pw_1")]
    qw2sb = v16("qw2")
    sl21 = v16("sl21")
    tw1b = v16("tw1b")
    qw1b = v16("qw1b")
    id13h = v16("id13h")
    bb2sb = vf("bb2t")
    bb3sb = vf("bb3t")
    tb2sb = vf("tb2t")
    c13sb = vf("c13")
    rw13sb = vf("rw13")
    id13sb = vf("id13")
    zero1 = consts.tile([128, 1], F32)
    nc.vector.memset(zero1, 0.0)

    state, stac16w, outdr = t["state"], t["stac16w"], t["out"]

    # PE p-state warmup: dummy matmuls on a zeroed tile while the first
    # input DMAs land, so block 0 starts at full clock.
    wu_l = consts.tile([4, 128], F16, name="wu_l")
    wu_r = consts.tile([4, NB], F16, name="wu_r")
    nc.vector.memset(wu_l, 0.0)
    nc.vector.memset(wu_r, 0.0)
    wu_ps = ps_mm.tile([128, NB], F32, tag="mm", bufs=7)
    for _ in range(40):
        nc.tensor.matmul(wu_ps, wu_l, wu_r, start=True, stop=True)

    ablk = {}

    def stage_a(blk):
        r0 = blk * NB
        st_ac = sb_in.tile([128, 4, SD], F32, tag="st_ac", bufs=4)
        nc.sync.dma_start(
            out=st_ac,
            in_=state[r0:r0 + NB, :].rearrange("(c p) d -> p c d", p=128))
        # 6 XBAR transposes: chunk 0 = features, 1-5 = enc channel groups
        stacT = sb_in.tile([128, NB], F16, tag="stacT", bufs=3)
        nc.sync.dma_start(out=stacT, in_=stac16w[r0:r0 + NB, 0:128],
                          transpose=True)
        srep = []
        for c in range(5):
            s_ = sb_in.tile([128, NB], F16, tag=f"srep{c}", bufs=3,
                            name=f"srep{c}")
            eng = nc.scalar if c % 2 else nc.sync
            eng.dma_start(
                out=s_,
                in_=stac16w[r0:r0 + NB, 128 * (c + 1):128 * (c + 2)],
                transpose=True)
            srep.append(s_)

        # q = dist^2 (K=21 fp16 matmul; |s|^2, |pos|^2 folded via const rows)
        q_ps = ps_mm.tile([128, NB], F32, tag="mm", bufs=7)
        nc.tensor.matmul(q_ps, sl21, stacT[0:21, :], start=True, stop=True)
        qs = sb_sm.tile([128, NB], F16, tag="qs", bufs=3)
        nc.scalar.activation(out=qs, in_=q_ps, func=AF.Relu,
                             bias=zero1[:, 0:1], scale=1.0)

        # dist = q * rsqrt(q): fp16 magic seed + 1 Newton step
        r = sb_sm.tile([128, NB], F16, tag="r", bufs=3)
        y = sb_sm.tile([128, NB], F16, tag="y", bufs=3)
        u = sb_sm.tile([128, NB], F16, tag="u", bufs=3)
        nc.vector.tensor_scalar(
            out=r.bitcast(I16), in0=qs.bitcast(I16), scalar1=1, scalar2=None,
            op0=ALU.logical_shift_right)
        nc.vector.tensor_scalar(
            out=r.bitcast(I16), in0=r.bitcast(I16), scalar1=-1,
            scalar2=0x59BA, op0=ALU.mult, op1=ALU.add)
        nc.gpsimd.tensor_mul(y, qs, r)
        nc.gpsimd.tensor_mul(u, y, r)
        nc.gpsimd.tensor_scalar(out=u, in0=u, scalar1=-0.5, scalar2=1.5,
                                op0=ALU.mult, op1=ALU.add)
        nc.gpsimd.tensor_mul(y, y, u)   # y = dist

        w16 = sb_in.tile([128, NB], F16, tag="w16", bufs=3)
        nc.scalar.activation(out=w16, in_=y, func=AF.Exp,
                             bias=zero1[:, 0:1], scale=-2.0)

        # enc = srep * w[p%32] on gpsimd (SBUF only); chunk 5: rows 0-31
        # are channels (*w), row 32 is the ones bias channel (copied).
        enc = []
        for c in range(4):
            et = sb_in.tile([128, NB], F16, tag=f"enc{c}", bufs=3,
                            name=f"enc{c}")
            nc.gpsimd.tensor_mul(et, srep[c], w16)
            enc.append(et)
        etC = sb_in.tile([33, NB], F16, tag="encC", bufs=3)
        nc.gpsimd.tensor_mul(etC[0:32, :], srep[4][0:32, :], w16[0:32, :])
        nc.vector.tensor_copy(etC[32:33, :], srep[4][32:33, :])
        enc.append(etC)
        ablk[blk] = dict(st_ac=st_ac, stacT=stacT, enc=enc)

    def stage_b(blk):
        st = ablk[blk]
        enc, stacT = st["enc"], st["stacT"]

        def relu_to(eng, dst, ps, bias_col=None):
            if eng == "act":
                nc.scalar.activation(
                    out=dst, in_=ps, func=AF.Relu,
                    bias=zero1[:, 0:1] if bias_col is None else bias_col,
                    scale=1.0)
            elif bias_col is None:
                nc.vector.tensor_scalar_max(dst, ps, 0.0)
            else:
                nc.vector.tensor_scalar(out=dst, in0=ps, scalar1=bias_col,
                                        scalar2=0.0, op0=ALU.add,
                                        op1=ALU.max)

        # ---- branch L1: 544 -> 1024 (5 fp16 chunks; bias pre-folded) ----
        h1 = [sb_act.tile([128, NB], F16, tag=f"h1_{m}", bufs=2,
                          name=f"h1_{m}") for m in range(8)]
        for m in range(8):
            ps = ps_mm.tile([128, NB], F32, tag="mm", bufs=7)
            for k in range(5):
                nc.tensor.matmul(ps, w1sb[k][:, m * 128:(m + 1) * 128],
                                 enc[k], start=(k == 0), stop=(k == 4))
            relu_to(RELU1_ENG[m], h1[m], ps)

        # ---- branch L2: 1024 -> 512 ----
        h2 = [sb_act.tile([128, NB], F16, tag=f"h2_{m}", bufs=2,
                          name=f"h2_{m}") for m in range(4)]
        for m in range(4):
            ps = ps_mm.tile([128, NB], F32, tag="mm", bufs=7)
            for k in range(8):
                nc.tensor.matmul(ps, w2sb[k][:, m * 128:(m + 1) * 128],
                                 h1[k], start=(k == 0), stop=(k == 7))
            relu_to(RELU2_ENG[m], h2[m], ps, bb2sb[:, m:m + 1])

        # ---- trunk: tanh(pos@tw1+tb1) [bias folded], tanh(.@tw2+tb2) ----
        tt = sb_act.tile([128, 2, NB], F16, tag="tt", bufs=2)
        for i_ in range(2):
            tt_ps = ps_mm.tile([128, NB], F32, tag="mm", bufs=7)
            nc.tensor.matmul(tt_ps, tw1b[:, 128 * i_:128 * (i_ + 1)],
                             stacT[0:4, :], start=True, stop=True)
            nc.scalar.activation(out=tt[:, i_, :], in_=tt_ps, func=AF.Tanh,
                                 bias=zero1[:, 0:1], scale=1.0)
        trunk = []
        for m in range(2):
            ps = ps_mm.tile([128, NB], F32, tag="mm", bufs=7)
            for k in range(2):
                nc.tensor.matmul(ps, tw2sb[k][:, m * 128:(m + 1) * 128],
                                 tt[:, k, :], start=(k == 0), stop=(k == 1))
            tm = sb_act.tile([128, NB], F16, tag="trunk", bufs=3)
            nc.scalar.activation(out=tm, in_=ps, func=AF.Tanh,
                                 bias=tb2sb[:, m:m + 1], scale=1.0)
            trunk.append(tm)

        # ---- qnet hidden: relu(pos@qw1+qb1) [bias folded] ----
        ps = ps_mm.tile([128, NB], F32, tag="mm", bufs=7)
        nc.tensor.matmul(ps, qw1b, stacT[0:4, :], start=True, stop=True)
        bq = sb_act.tile([128, NB], F16, tag="bq", bufs=2)
        nc.scalar.activation(out=bq, in_=ps, func=AF.Relu,
                             bias=zero1[:, 0:1], scale=1.0)

        # ---- branch L3 (+bias) fused with interaction multiply ----
        inter = []
        for m in range(2):
            ps = ps_mm.tile([128, NB], F32, tag="mm", bufs=7)
            for k in range(4):
                nc.tensor.matmul(ps, w3sb[k][:, m * 128:(m + 1) * 128],
                                 h2[k], start=(k == 0), stop=(k == 3))
            im = sb_act.tile([128, NB], F16, tag=f"inter{m}", bufs=2,
                             name=f"inter{m}")
            nc.vector.scalar_tensor_tensor(
                out=im, in0=ps, scalar=bb3sb[:, m:m + 1], in1=trunk[m],
                op0=ALU.add, op1=ALU.mult)
            inter.append(im)

        # ---- tail: delta^T + bias_out^T accumulated in one psum ----
        tail_full = ps_mm.tile([128, NB], F32, tag="mm", bufs=7)
        tail_ps = tail_full[0:SD, :]
        nc.tensor.matmul(tail_ps, pwsb[0], inter[0], start=True, stop=False)
        nc.tensor.matmul(tail_ps, pwsb[1], inter[1], start=False, stop=False)
        nc.tensor.matmul(tail_ps, qw2sb, bq, start=False, stop=True)
        combT = sb_sm.tile([16, NB], F16, tag="combT", bufs=3)
        nc.vector.tensor_scalar(
            out=combT[0:SD, :], in0=tail_ps, scalar1=rw13sb[:, 0:1],
            scalar2=c13sb[:, 0:1], op0=ALU.mult, op1=ALU.add)
        ablk[blk]["combT"] = combT

    def stage_c(blk):
        r0 = blk * NB
        st = ablk.pop(blk)
        st_ac, combT = st["st_ac"], st["combT"]
        trt = ps_tr.tile([128, 64], F16, tag="tr", bufs=1)
        nxt = sb_sm.tile([128, 4, SD], F32, tag="nxt", bufs=2)
        sq = sb_sm.tile([128, 4, 4], F32, tag="sq", bufs=2)
        for c in range(4):
            tr_ps = trt[:, 16 * c:16 * c + SD]
            nc.tensor.transpose(tr_ps, combT[0:SD, c * 128:(c + 1) * 128],
                                id13h)
            nc.vector.tensor_add(nxt[:, c, :], tr_ps, st_ac[:, c, :])
            nc.vector.tensor_mul(sq[:, c, :], nxt[:, c, 3:7], nxt[:, c, 3:7])
        qn = sb_sm.tile([128, 4], F32, tag="qn", bufs=2)
        nc.vector.reduce_sum(out=qn.rearrange("p (c o) -> p c o", o=1),
                             in_=sq, axis=AX.X)
        rq = sb_sm.tile([128, 4], F32, tag="rq", bufs=2)
        uq = sb_sm.tile([128, 4], F32, tag="uq", bufs=2)
        yq = sb_sm.tile([128, 4], F32, tag="yq", bufs=2)
        nc.vector.tensor_scalar(
            out=rq.bitcast(I32), in0=qn.bitcast(I32), scalar1=1, scalar2=None,
            op0=ALU.arith_shift_right)
        nc.vector.tensor_scalar(
            out=rq.bitcast(I32), in0=rq.bitcast(I32), scalar1=-1,
            scalar2=0x5F3759DF, op0=ALU.mult, op1=ALU.add)
        for it in range(2):
            nc.gpsimd.tensor_mul(yq, qn, rq)
            nc.gpsimd.tensor_mul(uq, yq, rq)
            nc.gpsimd.tensor_scalar(out=uq, in0=uq, scalar1=-0.5, scalar2=1.5,
                                    op0=ALU.mult, op1=ALU.add)
            nc.gpsimd.tensor_mul(rq, rq, uq)
        outt = sb_sm.tile([128, 4, SD], F32, tag="outt", bufs=2)
        nc.gpsimd.tensor_copy(outt, nxt)
        for c in range(4):
            nc.gpsimd.tensor_scalar_mul(
                outt[:, c, 3:7], nxt[:, c, 3:7], rq[:, c:c + 1])
        out_dst = outdr[r0:r0 + NB, :].rearrange("(c p) d -> p c d", p=128)
        nc.sync.dma_start(out=out_dst, in_=outt)

    # software-pipelined emission: A two blocks ahead of B/C
    stage_a(0)
    if nblk > 1:
        stage_a(1)
    for blk in range(nblk):
        stage_b(blk)
        if blk + 2 < nblk:
            stage_a(blk + 2)
        stage_c(blk)
    stack.close()


def _host_prep(inputs):
    """Precompute fp16 weight blob and the wide replicated feature matrix."""
    f = lambda x: np.ascontiguousarray(np.asarray(x, dtype=np.float32))
    sl = f(inputs["sensor_locations"])            # [32, 3]
    pidx = np.arange(128) % NS

    sl21 = np.zeros((21, 128), np.float32)
    sl21[0:3, :] = -2.0 * sl[pidx].T
    sl21[3, :] = np.square(sl).sum(1)[pidx]
    sl21[18:21, :] = 1.0
    tw1b = np.concatenate([f(inputs["tw1"]), f(inputs["tb1"])[None, :]], 0)
    qw1b = np.concatenate([f(inputs["qw1"]), f(inputs["qb1"])[None, :]], 0)

    # enc channel ch = j*32 + s  <-  original bw1 row s*17 + j
    ch = np.arange(544)
    w1p = f(inputs["bw1"])[(ch % NS) * J + ch // NS, :]        # [544, 1024]

    c16 = {"sl21": sl21, "tw1b": tw1b, "qw1b": qw1b,
           "id13h": np.eye(SD, dtype=np.float32)}
    for k in range(4):
        c16[f"w1_{k}"] = w1p[k * 128:(k + 1) * 128]
    w1c = np.zeros((33, H1), np.float32)
    w1c[0:32] = w1p[512:544]
    w1c[32] = f(inputs["bb1"])
    c16["w1_4"] = w1c
    w2 = f(inputs["bw2"]); w3 = f(inputs["bw3"]); tw2 = f(inputs["tw2"])
    for k in range(8):
        c16[f"w2_{k}"] = w2[k * 128:(k + 1) * 128]
    for k in range(4):
        c16[f"w3_{k}"] = w3[k * 128:(k + 1) * 128]
    for k in range(2):
        c16[f"tw2_{k}"] = tw2[k * 128:(k + 1) * 128]
    pw = f(inputs["pw"])
    c16["pw_0"] = pw[0:128]
    c16["pw_1"] = pw[128:256]
    c16["qw2"] = f(inputs["qw2"])

    def tb(b, nm):
        return np.ascontiguousarray(f(b).reshape(nm, 128).T)

    rw = np.float32(np.asarray(inputs["residual_weight"]))
    cf = {
        "bb2t": tb(inputs["bb2"], 4), "bb3t": tb(inputs["bb3"], 2),
        "tb2t": tb(inputs["tb2"], 2),
        "c13": (rw * (f(inputs["pb"]) + f(inputs["qb2"]))).reshape(SD, 1),
        "rw13": np.full((SD, 1), rw, np.float32),
        "id13": np.eye(SD, dtype=np.float32),
    }

    blob16 = np.zeros((128, C16W), np.float16)
    for name, (o, p, w) in C16.items():
        blob16[0:p, o:o + w] = c16[name].astype(np.float16)
    blobf = np.zeros((128, CFW), np.float32)
    for name, (o, p, w) in CF.items():
        blobf[0:p, o:o + w] = cf[name]

    # stac16w: fp16 features + host-replicated enc channels
    st = f(inputs["state"]); ac = f(inputs["action"])
    B = st.shape[0]
    feat = np.zeros((B, 21), np.float32)
    feat[:, 0:3] = st[:, 0:3]
    feat[:, 3] = 1.0
    feat[:, 4:14] = st[:, 3:13]
    feat[:, 14:18] = ac
    feat[:, 18:21] = np.square(st[:, 0:3])
    stac16w = np.zeros((B, WC), np.float16)
    stac16w[:, 0:21] = feat
    # channels ch = j*32+s -> column 128+ch; value = feature j
    jvals = np.concatenate([st, ac], axis=1).astype(np.float16)  # [B, 17]
    stac16w[:, 128:672] = np.repeat(jvals, NS, axis=1)
    stac16w[:, 672] = 1.0
    return dict(blob16=blob16, blobf=blobf), stac16w


def _core_inputs(inputs, common=None):
    """Build the 8 per-core input maps from the full problem inputs."""
    if common is None:
        common, stac16w = _host_prep(inputs)
    else:
        common, stac16w = common
    state = np.ascontiguousarray(np.asarray(inputs["state"], np.float32))
    in_maps = []
    for i in range(N_CORES):
        m = dict(common)
        m["state"] = state[i * RPC:(i + 1) * RPC]
        m["stac16w"] = stac16w[i * RPC:(i + 1) * RPC]
        in_maps.append(m)
    return in_maps


_NC_CACHE = {}


def _get_nc(rpc=RPC):
    if rpc not in _NC_CACHE:
        _NC_CACHE[rpc] = build_nc(rpc)
    return _NC_CACHE[rpc]


def kernel(**inputs):
    from concourse.bass_utils import run_bass_kernel_spmd

    nc = _get_nc()
    in_maps = _core_inputs(inputs)
    res = run_bass_kernel_spmd(nc, in_maps, list(range(N_CORES)))
    return np.concatenate([r["out"] for r in res.results], axis=0)
